# revision 51
# baseline (speedup 1.0000x reference)
"""MLA (Multi-Head Latent Attention) Bass kernel for 8 Trainium2 NeuronCores.

Sharding: 8 cores = 2 (batch) x 4 (head groups). Core c -> batch c//4,
group g=c%4 owning heads {2g, 2g+1, 2g+8, 2g+9} (paired h/h+8 so the
rotate-half RoPE over d_model=2048 stays core-local).

q path goes through the latent: each core computes q_latent for its
512-token tile (contraction 2048 -> 1536), the 4-core batch group
allgathers latq (split in two halves so the collective overlaps
compute), and each core projects its 4 head strips from the gathered
latent (contraction 1536). This replaces the old replicated
Wq_down@Wq_up fold (which cost 6.4 GFLOP/core + a DRAM round trip).

kv path: latkv + the shared k_rope are computed token-sharded and
gathered in ONE collective (krope packed as a 5th row-block of the
latkv shard buffer).

Attention scores are computed in [k, q] layout; exp runs on the scalar
engine straight out of PSUM (scores are bounded, no max subtraction).
The softmax denominator is hybrid: off-diagonal exp chunks accumulate
on the vector engine (exsum), diagonal chunks go through the ones
matmul on the PE, and one final ones@exsum matmul folds the rest in.
1/denom is folded into the attention-output scaling.

Each core computes a partial out^T = (attn_out_g @ Wout[rows_g]).T for
its 4 heads in bf16; the host sums the 4 partials per batch (fp32) and
transposes. bout is added on-device by the g==0 cores only.
"""
import os
import sys

if "/opt/trn_rl_repo" not in sys.path:
    sys.path.insert(0, "/opt/trn_rl_repo")

import numpy as np

D_MODEL = 2048
Q_LAT = 1536
KV_LAT = 512
NUM_HEADS = 16
HD = 128
B, S = 2, 2048
SCALE = 1.0 / np.sqrt(2.0 * HD)  # 1/16

QT = 512          # query tile width (matmul free dim)
NQT = S // QT     # 4
NC_DM = D_MODEL // 128   # 16 chunks of the model dim
NC_QL = Q_LAT // 128     # 12
NC_KV = KV_LAT // 128    # 4
NKC = S // 128           # 16 key chunks

_CACHE = {}
LAST_RESULT = None


def _strip_cols(g):
    """Global column starts (width 128) of the 4 local head strips, in local
    order [2g, 2g+1, 2g+8, 2g+9]."""
    return [256 * g, 256 * g + 128, 1024 + 256 * g, 1024 + 256 * g + 128]


def _cc_on_stream(nc, mybir, groups, in_ap, out_ap, stream_id=0):
    """AllGather like nc.gpsimd.collective_compute, but with an explicit CC
    stream_id so two collectives can run concurrently."""
    from concourse.replica_groups import filter_and_check_groups

    nc.has_collectives = True
    groups = filter_and_check_groups(nc.num_devices, groups)
    eng = nc.gpsimd
    return eng.add_instruction(
        mybir.InstCollectiveCompute(
            name=f"I-{nc.next_id()}",
            kind="AllGather",
            op=mybir.AluOpType.bypass,
            replica_groups=groups,
            ins=[eng.lower_ap(in_ap)],
            outs=[eng.lower_ap(out_ap)],
            unique_tensors="No",
            cc_dim="Partition",
            stream_id=stream_id,
        )
    )


def _build_bass():
    from concourse import bacc, mybir
    from concourse.tile import TileContext

    f32 = mybir.dt.float32
    bf16 = mybir.dt.bfloat16
    AF = mybir.ActivationFunctionType

    nc = bacc.Bacc("TRN2", target_bir_lowering=False, debug=False, num_devices=8)

    def inp(name, shape, dt=bf16):
        return nc.dram_tensor(name, list(shape), dt, kind="ExternalInput")

    # x packed partition-contiguous: [p][c*QT+k] per token tile
    xq_p = inp("xq_p", (NQT, 128, NC_DM * QT))   # full batch xqT, tile-major
    xk_p = inp("xk_p", (128, NC_DM * QT))        # this core's k-tile of xkT
    # Wq_down^T packed partition-major for the fold: [p=lat][c][l*128+f(dm)]
    wq_downT = inp("wq_downT", (128, NC_DM, NC_QL * 128))
    wkv_down = inp("wkv_down", (NC_KV, 128, NC_DM * 128))  # [s][p=dm][c*128+f]
    wk_rope = inp("wk_rope", (128, NC_DM * 128))           # [p=dm][c*128+f]
    # up-proj slices: [p=lat-within-chunk][l(lat-chunk)*512 + f(4 strips x 128)]
    wq_up = inp("wq_up", (128, NC_QL * 512))
    wq_rope = inp("wq_rope", (128, NC_QL * 512))
    wk_up = inp("wk_up", (4, 128, NC_KV * 128))            # [strip][p=lat][c*128+f]
    wv_up = inp("wv_up", (128, NC_KV * 512))               # [p=lat][c*512+f]
    wout = inp("wout", (128, 64 * 128))                    # [p][(m*4+h)*128+f]
    cos_q = inp("cos_q", (2, 128, S))                      # [block j][d][q], bf16
    sin_q = inp("sin_q", (2, 128, S))
    cos_k = inp("cos_k", (64, QT))
    sin_k = inp("sin_k", (64, QT))
    masks = inp("masks", (128, 4 * QT))                    # [kl][(o*QT)+ql]
    ones = inp("ones", (128, 128))
    bias = inp("bias", (128, NC_DM), f32)                  # [p][m]

    outT = nc.dram_tensor("outT", [D_MODEL, S], bf16, kind="ExternalOutput")

    # tiny dummy collective to absorb the one-time CC setup latency at t=0;
    # spans ALL 8 cores so both batch groups sync before the pair exchange
    warm_sh_d = nc.dram_tensor("warm_sh_d", [128, 4], bf16, kind="Internal")
    warm_g_d = nc.dram_tensor("warm_g_d", [8, 128, 4], bf16, kind="Internal")
    G_ALL = [[0, 1, 2, 3, 4, 5, 6, 7]]
    # latkv (4 blocks) + krope (block 4) shard + gather, one collective;
    # partition-contiguous layout so the DMAs are 128-descriptor
    lkv_sh_d = nc.dram_tensor("lkv_sh_d", [128, 5 * QT], bf16, kind="Internal")
    lkv_g_d = nc.dram_tensor("lkv_g_d", [4, 128, 5 * QT], bf16, kind="Internal")
    G_BATCH = [[0, 1, 2, 3], [4, 5, 6, 7]]

    with TileContext(nc) as tc:
        with tc.tile_pool(name="kvres", bufs=1) as kvres:
            # resident across phases: k/v for all 2048 keys
            kproj_sb = kvres.tile([128, 4, S], bf16)
            krope_sb = kvres.tile([128, S], bf16)
            v_sb = kvres.tile([128, NKC, 512], bf16)
            weff_sb = kvres.tile([128, 2, NC_DM, 512], bf16)

            # ----- P1: latkv + krope for this core's k-tile; P2: latq -----
            with tc.tile_pool(name="p1", bufs=1) as p1, \
                 tc.tile_pool(name="p1t", bufs=2) as p1t, \
                 tc.tile_pool(name="p2", bufs=1) as p2, \
                 tc.tile_pool(name="p12ps", bufs=3, space="PSUM") as p12ps:
                # warmup collective: no data deps, fires immediately
                warm_t = p1t.tile([128, 4], bf16, tag="warm")
                nc.vector.memset(warm_t, 0.0)
                nc.sync.dma_start(out=warm_sh_d.ap(), in_=warm_t)
                nc.gpsimd.collective_compute(
                    "AllGather", mybir.AluOpType.bypass, replica_groups=G_ALL,
                    ins=[warm_sh_d.ap()], outs=[warm_g_d.ap()])

                # critical-path loads first: xk tile + kv-down weights,
                # quarter-granular so the first matmuls start early
                xk_t = p1.tile([128, NC_DM, QT], bf16)
                wkv_sb = p1.tile([128, NC_KV * NC_DM * 128], bf16)
                xk_p_v = xk_p.ap().rearrange("p (c k) -> p c k", c=NC_DM)
                for q4 in range(4):
                    nc.sync.dma_start(
                        out=xk_t[:, 4 * q4:4 * q4 + 4, :],
                        in_=xk_p_v[:, 4 * q4:4 * q4 + 4, :])
                    nc.sync.dma_start(
                        out=wkv_sb[:, (0 * NC_DM + 4 * q4) * 128:(0 * NC_DM + 4 * q4 + 4) * 128],
                        in_=wkv_down.ap()[0][:, 4 * q4 * 128:(4 * q4 + 4) * 128])
                for s in range(1, NC_KV):
                    nc.sync.dma_start(
                        out=wkv_sb[:, s * NC_DM * 128:(s + 1) * NC_DM * 128],
                        in_=wkv_down.ap()[s])
                wkr_sb = p1.tile([128, NC_DM * 128], bf16)
                nc.sync.dma_start(out=wkr_sb, in_=wk_rope.ap())
                cosk_sb = p1.tile([64, QT], bf16)
                sink_sb = p1.tile([64, QT], bf16)
                nc.sync.dma_start(out=cosk_sb, in_=cos_k.ap())
                nc.sync.dma_start(out=sink_sb, in_=sin_k.ap())
                # prefetch the fold's inputs behind them (up-slices first —
                # every fold chunk contracts over all of them; wq_downT in
                # quarters so the first fold chunks start before 6.3MB lands)
                wqu_sb = p2.tile([128, NC_QL * 512], bf16)
                nc.sync.dma_start(out=wqu_sb, in_=wq_up.ap())
                wqdT_sb = p2.tile([128, NC_DM, NC_QL * 128], bf16)
                nc.sync.dma_start(
                    out=wqdT_sb[:, 0:4, :], in_=wq_downT.ap()[:, 0:4, :])
                wqr_sb = p2.tile([128, NC_QL * 512], bf16)
                nc.sync.dma_start(out=wqr_sb, in_=wq_rope.ap())
                for q4 in range(1, 4):
                    nc.sync.dma_start(
                        out=wqdT_sb[:, 4 * q4:4 * q4 + 4, :],
                        in_=wq_downT.ap()[:, 4 * q4:4 * q4 + 4, :])

                # latkv + krope shard -> one packed buffer [128][5][QT]
                lkv_sh = p1.tile([128, 5, QT], bf16)
                for s in range(NC_KV):
                    ps = p12ps.tile([128, QT], f32, tag="ps")
                    for c in range(NC_DM):
                        nc.tensor.matmul(
                            ps, wkv_sb[:, (s * NC_DM + c) * 128:(s * NC_DM + c + 1) * 128],
                            xk_t[:, c, :], start=(c == 0), stop=(c == NC_DM - 1))
                    nc.scalar.copy(out=lkv_sh[:, s, :], in_=ps)
                ps = p12ps.tile([128, QT], f32, tag="ps")
                for c in range(NC_DM):
                    nc.tensor.matmul(
                        ps, wkr_sb[:, c * 128:(c + 1) * 128],
                        xk_t[:, c, :], start=(c == 0), stop=(c == NC_DM - 1))
                krraw = p1t.tile([128, QT], f32, tag="krraw")
                nc.scalar.copy(out=krraw, in_=ps)
                krb = p1t.tile([64, QT], f32, tag="krb")
                nc.sync.dma_start(out=krb, in_=krraw[64:128, :])
                t1 = p1t.tile([64, QT], f32, tag="krt1")
                t2 = p1t.tile([64, QT], f32, tag="krt2")
                nc.vector.tensor_mul(t1, krraw[0:64, :], cosk_sb)
                nc.vector.tensor_mul(t2, krb, sink_sb)
                nc.vector.tensor_sub(lkv_sh[0:64, 4, :], t1, t2)
                obot = p1t.tile([64, QT], bf16, tag="krob")
                nc.vector.tensor_mul(t1, krb, cosk_sb)
                nc.vector.tensor_mul(t2, krraw[0:64, :], sink_sb)
                nc.vector.tensor_add(obot, t1, t2)
                nc.sync.dma_start(out=lkv_sh[64:128, 4, :], in_=obot)
                nc.sync.dma_start(
                    out=lkv_sh_d.ap().rearrange("p (s k) -> p s k", s=5),
                    in_=lkv_sh)
                nc.gpsimd.collective_compute(
                    "AllGather", mybir.AluOpType.bypass, replica_groups=G_BATCH,
                    ins=[lkv_sh_d.ap()], outs=[lkv_g_d.ap()])

                # fold both kinds of Weff = Wq_down @ W{up,rope}[:, strips]
                # straight into resident SBUF (no collective, no round trip)
                for c in range(NC_DM):
                    for k2, wup in ((0, wqu_sb), (1, wqr_sb)):
                        ps = p12ps.tile([128, QT], f32, tag="ps")
                        for l in range(NC_QL):
                            nc.tensor.matmul(
                                ps, wqdT_sb[:, c, l * 128:(l + 1) * 128],
                                wup[:, l * 512:(l + 1) * 512],
                                start=(l == 0), stop=(l == NC_QL - 1))
                        nc.scalar.copy(out=weff_sb[:, k2, c, :], in_=ps)

            # ----- P3: k_proj + V + krope from gathered latkv --------------
            with tc.tile_pool(name="p3", bufs=1) as p3, \
                 tc.tile_pool(name="p3ps", bufs=6, space="PSUM") as p3ps:
                latkv_sb = p3.tile([128, 4, 5, QT], bf16)
                for kt in range(4):
                    nc.sync.dma_start(
                        out=latkv_sb[:, kt],
                        in_=lkv_g_d.ap()[kt].rearrange("p (s k) -> p s k", s=5))
                    nc.sync.dma_start(
                        out=krope_sb[:, kt * QT:(kt + 1) * QT],
                        in_=lkv_g_d.ap()[kt][:, 4 * QT:5 * QT])
                wku_sb = p3.tile([128, 4 * NC_KV * 128], bf16)
                for s in range(4):
                    nc.sync.dma_start(
                        out=wku_sb[:, s * NC_KV * 128:(s + 1) * NC_KV * 128],
                        in_=wk_up.ap()[s])
                wvu_sb = p3.tile([128, NC_KV * 512], bf16)
                nc.sync.dma_start(out=wvu_sb, in_=wv_up.ap())
                for kt in range(4):
                    for s in range(4):
                        ps = p3ps.tile([128, QT], f32, tag="ps")
                        for c in range(NC_KV):
                            nc.tensor.matmul(
                                ps, wku_sb[:, (s * NC_KV + c) * 128:(s * NC_KV + c + 1) * 128],
                                latkv_sb[:, kt, c, :], start=(c == 0), stop=(c == NC_KV - 1))
                        nc.scalar.copy(out=kproj_sb[:, s, kt * QT:(kt + 1) * QT], in_=ps)
                    for kc in range(4):
                        ps = p3ps.tile([128, 512], f32, tag="ps")
                        for c in range(NC_KV):
                            nc.tensor.matmul(
                                ps, latkv_sb[:, kt, c, kc * 128:(kc + 1) * 128],
                                wvu_sb[:, c * 512:(c + 1) * 512],
                                start=(c == 0), stop=(c == NC_KV - 1))
                        nc.vector.tensor_copy(out=v_sb[:, kt * 4 + kc, :], in_=ps)

            # ----- tiles: q-strips (A2) + attention + out-proj per q-tile --
            with tc.tile_pool(name="cw", bufs=1) as cw, \
                 tc.tile_pool(name="clq", bufs=2) as clq, \
                 tc.tile_pool(name="cqn", bufs=2) as cqn, \
                 tc.tile_pool(name="craw", bufs=2) as craw, \
                 tc.tile_pool(name="ct", bufs=2) as ct, \
                 tc.tile_pool(name="cex", bufs=4) as cex, \
                 tc.tile_pool(name="ces", bufs=2) as ces, \
                 tc.tile_pool(name="cat", bufs=2) as cat, \
                 tc.tile_pool(name="co", bufs=3) as co, \
                 tc.tile_pool(name="ps2", bufs=2, space="PSUM") as ps2, \
                 tc.tile_pool(name="bps", bufs=3, space="PSUM") as bps, \
                 tc.tile_pool(name="bpd", bufs=1, space="PSUM") as bpd, \
                 tc.tile_pool(name="bpv", bufs=2, space="PSUM") as bpv:
                cosq_sb = cw.tile([128, 2, S], bf16)
                sinq_sb = cw.tile([128, 2, S], bf16)
                for j in range(2):
                    nc.sync.dma_start(out=cosq_sb[:, j, :], in_=cos_q.ap()[j])
                    nc.sync.dma_start(out=sinq_sb[:, j, :], in_=sin_q.ap()[j])
                masks_sb = cw.tile([128, 4 * QT], bf16)
                nc.sync.dma_start(out=masks_sb, in_=masks.ap())
                ones_sb = cw.tile([128, 128], bf16)
                nc.sync.dma_start(out=ones_sb, in_=ones.ap())
                wout_sb = cw.tile([128, 64 * 128], bf16)
                nc.sync.dma_start(out=wout_sb, in_=wout.ap())
                bias_sb = cw.tile([128, NC_DM], f32)
                nc.sync.dma_start(out=bias_sb, in_=bias.ap())

                for qt in range(NQT):
                    q0 = qt * QT
                    K = 4 * (qt + 1)   # causal: key chunks 0..K-1
                    nd = 4 * qt        # chunks < nd are fully below diagonal

                    # ---- A2: q_proj + q_rope strips from x @ Weff ----
                    xq_t = clq.tile([128, NC_DM, QT], bf16, tag="xq")
                    nc.sync.dma_start(
                        out=xq_t,
                        in_=xq_p.ap()[qt].rearrange("p (c k) -> p c k", c=NC_DM))
                    qn_t = cqn.tile([128, 8, QT], bf16, tag="qn")
                    for s in range(4):
                        ps = ps2.tile([128, QT], f32, tag="ps2")
                        for c in range(NC_DM):
                            nc.tensor.matmul(
                                ps, weff_sb[:, 0, c, s * 128:(s + 1) * 128],
                                xq_t[:, c, :], start=(c == 0), stop=(c == NC_DM - 1))
                        nc.scalar.copy(out=qn_t[:, 2 * s, :], in_=ps)
                    raws = []
                    for s in range(4):
                        ps = ps2.tile([128, QT], f32, tag="ps2")
                        for c in range(NC_DM):
                            nc.tensor.matmul(
                                ps, weff_sb[:, 1, c, s * 128:(s + 1) * 128],
                                xq_t[:, c, :], start=(c == 0), stop=(c == NC_DM - 1))
                        rw = craw.tile([128, QT], f32, tag=f"raw{s}")
                        nc.scalar.copy(out=rw, in_=ps)
                        raws.append(rw)
                    for j in range(2):
                        a, b = raws[j], raws[2 + j]
                        cj = cosq_sb[:, j, q0:q0 + QT]
                        sj = sinq_sb[:, j, q0:q0 + QT]
                        t1 = ct.tile([128, QT], bf16, tag=f"t1{j}")
                        t2 = ct.tile([128, QT], bf16, tag=f"t2{j}")
                        nc.vector.tensor_mul(t1, a, cj)
                        nc.vector.tensor_mul(t2, b, sj)
                        nc.vector.tensor_sub(qn_t[:, 2 * j + 1, :], t1, t2)
                        nc.vector.tensor_mul(t1, b, cj)
                        nc.vector.tensor_mul(t2, a, sj)
                        nc.vector.tensor_add(qn_t[:, 2 * (2 + j) + 1, :], t1, t2)

                    # ---- attention for this q tile ----
                    attn = cat.tile([128, 4, QT], bf16, tag="attn")
                    for h in range(4):
                        psd = bpd.tile([128, QT], f32, tag="psd")
                        psv = bpv.tile([128, QT], f32, tag="psv")
                        exsum = ces.tile([128, QT], f32, tag="exsum")

                        def consume(kc, ex):
                            # PE consumer of ex(kc); deferred one chunk so
                            # the exp->matmul latency hides behind the next
                            # chunk's score matmuls (PE executes in order)
                            nc.tensor.matmul(
                                psv, v_sb[:, kc, h * 128:(h + 1) * 128], ex,
                                start=(kc == 0), stop=(kc == K - 1),
                                skip_group_check=True)

                        prev = None
                        for kc in range(K):
                            pss = bps.tile([128, QT], f32, tag="pss")
                            nc.tensor.matmul(
                                pss, kproj_sb[:, h, kc * 128:(kc + 1) * 128],
                                qn_t[:, 2 * h, :], start=True, stop=False)
                            nc.tensor.matmul(
                                pss, krope_sb[:, kc * 128:(kc + 1) * 128],
                                qn_t[:, 2 * h + 1, :], start=False, stop=True)
                            ex = cex.tile([128, QT], bf16, tag="ex")
                            nc.scalar.activation(out=ex, in_=pss, func=AF.Exp,
                                                 scale=float(SCALE))
                            if kc >= nd:
                                # diagonal chunk: apply causal mask
                                o = kc - nd
                                nc.vector.tensor_mul(
                                    ex, ex, masks_sb[:, o * QT:(o + 1) * QT])
                            # accumulate exp on DVE (softmax denominator)
                            if kc == 0:
                                nc.vector.tensor_copy(out=exsum, in_=ex)
                            else:
                                nc.vector.tensor_add(exsum, exsum, ex)
                            if prev is not None:
                                consume(*prev)
                            prev = (kc, ex)
                        consume(*prev)
                        esb = cex.tile([128, QT], bf16, tag="esb")
                        nc.scalar.copy(out=esb, in_=exsum)
                        nc.tensor.matmul(psd, ones_sb, esb)
                        rec = ct.tile([128, QT], f32, tag="rec")
                        nc.vector.reciprocal_approx_fast(out=rec, in_=psd)
                        nc.vector.tensor_mul(attn[:, h, :], psv, rec)

                    # ---- output projection for this q tile ----
                    for m in range(NC_DM):
                        pso = ps2.tile([128, QT], f32, tag="ps2")
                        for h in range(4):
                            nc.tensor.matmul(
                                pso, wout_sb[:, (m * 4 + h) * 128:(m * 4 + h + 1) * 128],
                                attn[:, h, :], start=(h == 0), stop=(h == 3))
                        oc = co.tile([128, QT], bf16, tag="oc")
                        nc.vector.tensor_scalar_add(oc, pso, bias_sb[:, m:m + 1])
                        nc.sync.dma_start(
                            out=outT.ap()[m * 128:(m + 1) * 128, q0:q0 + QT], in_=oc)

    nc.finalize()
    return nc


def _host_pack(inputs):
    """Build the 8 per-core input maps from the full inputs."""
    import ml_dtypes
    bf16 = ml_dtypes.bfloat16

    xq = np.ascontiguousarray(inputs["inputs_q"], dtype=np.float32)
    xk = np.ascontiguousarray(inputs["inputs_k"], dtype=np.float32)
    Wq_down = np.asarray(inputs["Wq_down"], dtype=np.float32)
    Wkv_down = np.asarray(inputs["Wkv_down"], dtype=np.float32)
    Wq_up = np.asarray(inputs["Wq_up"], dtype=np.float32)
    Wk_up = np.asarray(inputs["Wk_up"], dtype=np.float32)
    Wv_up = np.asarray(inputs["Wv_up"], dtype=np.float32)
    Wq_rope = np.asarray(inputs["Wq_rope"], dtype=np.float32)
    Wk_rope = np.asarray(inputs["Wk_rope"], dtype=np.float32)
    Wout = np.asarray(inputs["Wout"], dtype=np.float32)
    bout = np.asarray(inputs["bout"], dtype=np.float32)

    def pack_lhs(W, n_strips, strip_starts, nchunks):
        # -> [n_strips, 128, nchunks*128]: [s][p][c*128+f]
        out = np.empty((n_strips, 128, nchunks * 128), dtype=bf16)
        for s in range(n_strips):
            blk = W[:, strip_starts[s]:strip_starts[s] + 128]  # [nchunks*128, 128]
            out[s] = blk.reshape(nchunks, 128, 128).transpose(1, 0, 2).reshape(128, -1).astype(bf16)
        return out

    # x packed partition-contiguous: [tile][p][c*QT+k] (= x[t*QT+k, c*128+p])
    xq_tiled = [np.ascontiguousarray(
        xq[b].reshape(NQT, QT, NC_DM, 128).transpose(0, 3, 2, 1)
        .reshape(NQT, 128, NC_DM * QT).astype(bf16)) for b in range(B)]
    xk_tiled = [np.ascontiguousarray(
        xk[b].reshape(NQT, QT, NC_DM, 128).transpose(0, 3, 2, 1)
        .reshape(NQT, 128, NC_DM * QT).astype(bf16)) for b in range(B)]

    # Wq_down^T packed partition-major for the fold: [p=lat][c][l*128+f(dm)]
    WqdT = np.ascontiguousarray(Wq_down.T)  # [Q_LAT, D_MODEL]
    wq_downT_p = np.ascontiguousarray(
        WqdT.reshape(NC_QL, 128, NC_DM, 128).transpose(1, 2, 0, 3)
        .reshape(128, NC_DM, NC_QL * 128).astype(bf16))

    wkv_down_p = pack_lhs(Wkv_down, NC_KV, [128 * s for s in range(NC_KV)], NC_DM)
    wk_rope_p = pack_lhs(Wk_rope, 1, [0], NC_DM)[0]

    # rope tables
    iq = np.arange(1024, dtype=np.float64)
    inv_q = 1.0 / (10000.0 ** (iq * 2.0 / D_MODEL))
    pos = np.arange(S, dtype=np.float64)
    ang_q = pos[:, None] * inv_q[None, :]          # [S, 1024]
    ik = np.arange(64, dtype=np.float64)
    inv_k = 1.0 / (10000.0 ** (ik * 2.0 / HD))
    ang_k = pos[:, None] * inv_k[None, :]          # [S, 64]
    cos_k_full = np.cos(ang_k).T.astype(bf16)  # [64, S]
    sin_k_full = np.sin(ang_k).T.astype(bf16)

    kl = np.arange(128)[:, None]
    ql = np.arange(QT)[None, :]
    masks = np.concatenate(
        [(kl + 128 * o <= ql).astype(np.float32) for o in range(4)], axis=1)
    masks = np.ascontiguousarray(masks.astype(bf16))
    ones = np.ones((128, 128), dtype=bf16)

    in_maps = []
    for c in range(8):
        b, g = divmod(c, 4)
        cols = _strip_cols(g)
        cols4 = np.concatenate([np.arange(cs, cs + 128) for cs in cols])

        # up-proj slices: [p=lat within chunk][l*512 + f]
        def pack_up(W):
            Wg = W[:, cols4]  # [Q_LAT, 512]
            return np.ascontiguousarray(
                Wg.reshape(NC_QL, 128, 512).transpose(1, 0, 2).reshape(128, -1).astype(bf16))
        wq_up_p = pack_up(Wq_up)
        wq_rope_p = pack_up(Wq_rope)
        wk_up_p = pack_lhs(Wk_up, 4, cols, NC_KV)
        Wv_g = Wv_up[:, cols4]                      # [512, 512]
        wv_up_p = np.ascontiguousarray(
            Wv_g.reshape(NC_KV, 128, 512).transpose(1, 0, 2).reshape(128, -1).astype(bf16))
        Wout_g = Wout[cols4, :].reshape(4, 128, NC_DM, 128)   # [h][p][m][f]
        wout_p = np.ascontiguousarray(
            Wout_g.transpose(1, 2, 0, 3).reshape(128, -1).astype(bf16))
        cos_q_p = np.empty((2, 128, S), dtype=bf16)
        sin_q_p = np.empty((2, 128, S), dtype=bf16)
        for j in range(2):
            idx = 256 * g + 128 * j + np.arange(128)
            cos_q_p[j] = np.cos(ang_q[:, idx]).T.astype(bf16)
            sin_q_p[j] = np.sin(ang_q[:, idx]).T.astype(bf16)
        bias_p = (bout if g == 0 else np.zeros_like(bout)).reshape(NC_DM, 128)
        bias_p = np.ascontiguousarray(bias_p.T)     # [128, m]

        k0 = QT * g
        in_maps.append({
            "xq_p": xq_tiled[b],
            "xk_p": xk_tiled[b][g],
            "wq_downT": wq_downT_p,
            "wkv_down": wkv_down_p, "wk_rope": wk_rope_p,
            "wq_up": wq_up_p, "wq_rope": wq_rope_p, "wk_up": wk_up_p,
            "wv_up": wv_up_p, "wout": wout_p,
            "cos_q": cos_q_p, "sin_q": sin_q_p,
            "cos_k": np.ascontiguousarray(cos_k_full[:, k0:k0 + QT]),
            "sin_k": np.ascontiguousarray(sin_k_full[:, k0:k0 + QT]),
            "masks": masks, "ones": ones, "bias": bias_p,
        })
    return in_maps


def kernel(**inputs):
    global LAST_RESULT
    from concourse.bass_utils import run_bass_kernel_spmd

    if "nc" not in _CACHE:
        _CACHE["nc"] = _build_bass()
    nc = _CACHE["nc"]

    in_maps = _host_pack(inputs)
    kwargs = {}
    if os.environ.get("KERNEL_TRACE"):
        try:
            sys.path.insert(0, os.path.dirname(os.path.abspath(__file__)))
            import axon_shim
            axon_shim.install()
            kwargs["trace"] = True
        except Exception:
            pass
    res = run_bass_kernel_spmd(nc, in_maps, core_ids=list(range(8)), **kwargs)
    LAST_RESULT = res

    out = np.empty((B, S, D_MODEL), dtype=np.float32)
    for b in range(B):
        acc = res.results[4 * b]["outT"].astype(np.float32)
        for g in range(1, 4):
            acc += res.results[4 * b + g]["outT"].astype(np.float32)
        out[b] = acc.T
    return out
